# revision 7
# baseline (speedup 1.0000x reference)
"""Trainium2 Bass kernel for a 6-layer GPT forward pass (logits + CE loss).

Sharding: 8-way sequence-parallel. B=2 batch rows x 4 chunks of 256 tokens.
Core c handles batch row c//4, token chunk c%4. Per layer, K/V are
all-gathered within each 4-core group; everything else is token-local.
The LM head + per-token sum-exp are computed locally per core; the host
assembles the full logits and the scalar loss.

Activations live channel-major ("x^T": [128 part, D/128, T_local]) in SBUF
for the whole trunk; matmul weights stream from HBM as bf16.
"""

import sys

for _p in ("/opt/trn_rl_repo", "/root/.axon_site/_ro/trn_rl_repo"):
    if _p not in sys.path:
        sys.path.insert(0, _p)

import numpy as np
import ml_dtypes

import concourse.bass as bass
import concourse.mybir as mybir
import concourse.tile as tile
from concourse import bacc
from concourse.bass_utils import run_bass_kernel_spmd
from concourse.masks import make_identity

F32 = mybir.dt.float32
BF16 = mybir.dt.bfloat16
I32 = mybir.dt.int32
AF = mybir.ActivationFunctionType
ALU = mybir.AluOpType
AX = mybir.AxisListType

# model dims (hardcoded per problem spec); V/L/VS are module globals so a
# small-config simulator harness can shrink them before building.
V, D, H, L, T, B = 32000, 1024, 16, 6, 1024, 2
VS = 500                 # vocab slice width for the LM head (V % VS == 0)
HD = D // H              # 64
P = 128
KO = D // P              # 8  (channel tiles)
NCORES = 8
GRP = 4                  # cores per batch row
TL = T // GRP            # 256 tokens per core
TT = TL // P             # 2  (token tiles)
DH = 4 * D               # 4096
EPS = 1e-5
GELU = AF.Gelu   # dev_sim swaps this for a sim-supported function

_CACHED = {}


def _ln_channel_major(nc, sb, ps, xT, ones_bf, eps_tile, out_bf, g_tile, b_tile):
    """LayerNorm over channels for channel-major xT [P, KO, TL] (fp32).

    Stats are computed replicated across partitions via ones-matmuls on a
    bf16 copy of x.  Writes normalized bf16 to out_bf [P, KO, TL].
    g_tile/b_tile: optional [P, KO] fp32 per-channel gain/bias.
    """
    x_bf = sb.tile([P, KO, TL], BF16, tag="ln_xbf", bufs=1, name="ln_xbf")
    nc.vector.tensor_copy(x_bf[:], xT[:])
    xsq_bf = sb.tile([P, KO, TL], BF16, tag="ln_xsq", bufs=1, name="ln_xsq")
    nc.scalar.square(xsq_bf[:], xT[:])

    sums = ps.tile([P, 512], F32, tag="ps", bufs=4, name="ln_sums")
    sumsq = ps.tile([P, 512], F32, tag="ps", bufs=4, name="ln_sumsq")
    for ko in range(KO):
        nc.tensor.matmul(sums[:, :TL], lhsT=ones_bf[:], rhs=x_bf[:, ko, :],
                         start=(ko == 0), stop=(ko == KO - 1))
    for ko in range(KO):
        nc.tensor.matmul(sumsq[:, :TL], lhsT=ones_bf[:], rhs=xsq_bf[:, ko, :],
                         start=(ko == 0), stop=(ko == KO - 1))

    mu = sb.tile([P, TL], F32, tag="ln_mu", bufs=2, name="ln_mu")
    nc.scalar.mul(mu[:], sums[:, :TL], 1.0 / D)
    # var = sumsq/D - mu^2 ; rstd = 1/sqrt(var+eps)
    var = sb.tile([P, TL], F32, tag="ln_var", bufs=2, name="ln_var")
    nc.vector.tensor_tensor(var[:], mu[:], mu[:], op=ALU.mult)
    nc.vector.scalar_tensor_tensor(
        out=var[:], in0=sumsq[:, :TL], scalar=1.0 / D, in1=var[:],
        op0=ALU.mult, op1=ALU.subtract)
    rstd = sb.tile([P, TL], F32, tag="ln_rstd", bufs=2, name="ln_rstd")
    nc.scalar.activation(rstd[:], var[:], AF.Sqrt, bias=eps_tile[:])
    nc.vector.reciprocal(rstd[:], rstd[:])

    xc = x_bf  # reuse: sums matmuls are the last reader of x_bf
    nc.vector.tensor_tensor(
        xc[:], xT[:], mu[:, None, :].to_broadcast([P, KO, TL]), op=ALU.subtract)
    if g_tile is not None:
        nc.vector.tensor_tensor(
            xc[:], xc[:], g_tile[:, :, None].to_broadcast([P, KO, TL]),
            op=ALU.mult)
    nc.vector.tensor_tensor(
        out_bf[:], xc[:], rstd[:, None, :].to_broadcast([P, KO, TL]),
        op=ALU.mult)
    if b_tile is not None:
        nc.vector.tensor_tensor(
            out_bf[:], out_bf[:], b_tile[:, :, None].to_broadcast([P, KO, TL]),
            op=ALU.add)


def build_nc(ln_trivial):
    """Build the SPMD program. ln_trivial: (ln1, ln2, lnf) flags for
    all-ones gain / zero bias, decided from actual inputs at build time."""
    NVS = V // VS
    assert V % VS == 0

    nc = bacc.Bacc("TRN2", target_bir_lowering=False, debug=False,
                   num_devices=NCORES)

    # ---- per-core DRAM inputs ----
    tok_emb = nc.dram_tensor("tok_emb", [V, D], BF16, kind="ExternalInput")
    idx = nc.dram_tensor("idx", [TL], I32, kind="ExternalInput")
    pos = nc.dram_tensor("pos", [TL, D], F32, kind="ExternalInput")
    maskT = nc.dram_tensor("maskT", [P, 2 * GRP, TL], BF16, kind="ExternalInput")
    wq = nc.dram_tensor("wq", [L, D, D], BF16, kind="ExternalInput")
    wk = nc.dram_tensor("wk", [L, D, D], BF16, kind="ExternalInput")
    wv = nc.dram_tensor("wv", [L, D, D], BF16, kind="ExternalInput")
    w1 = nc.dram_tensor("w1", [L, D, DH], BF16, kind="ExternalInput")
    w2 = nc.dram_tensor("w2", [L, DH, D], BF16, kind="ExternalInput")
    lnw = None
    if not all(ln_trivial):
        # rows: ln1_g(L), ln1_b(L), ln2_g(L), ln2_b(L), lnf_g, lnf_b
        lnw = nc.dram_tensor("lnw", [4 * L + 2, D], F32, kind="ExternalInput")
    lm_w = nc.dram_tensor("lm_w", [D, V], BF16, kind="ExternalInput")

    # ---- per-core DRAM outputs ----
    logits = nc.dram_tensor("logits", [TL, V], F32, kind="ExternalOutput")
    lse = nc.dram_tensor("lse", [P, TT], F32, kind="ExternalOutput")

    groups = [[0, 1, 2, 3], [4, 5, 6, 7]]

    with tile.TileContext(nc) as tc:
        with (
            tc.tile_pool(name="sb", bufs=2) as sb,
            tc.tile_pool(name="ps", bufs=4, space="PSUM") as ps,
            tc.tile_pool(name="dram", bufs=2, space="DRAM") as dram,
        ):
            ones_bf = sb.tile([P, P], BF16, tag="ones", bufs=1, name="ones")
            nc.vector.memset(ones_bf[:], 1.0)
            ident = sb.tile([P, P], F32, tag="ident", bufs=1, name="ident")
            make_identity(nc, ident[:])
            mask_sb = sb.tile([P, 2 * GRP, TL], BF16, tag="mask", bufs=1,
                              name="mask_sb")
            eps_tile = sb.tile([P, 1], F32, tag="eps", bufs=1, name="eps_tile")
            nc.vector.memset(eps_tile[:], EPS)
            nc.sync.dma_start(mask_sb[:], maskT[:, :, :])

            lnsb = None
            if lnw is not None:
                lnsb = sb.tile([4 * L + 2, P, KO], F32, tag="lnsb", bufs=1,
                               name="lnsb")
                nc.sync.dma_start(
                    lnsb[:], lnw[:, :].rearrange("n (ko p) -> n p ko", p=P))

            def ln_gb(kind, layer):
                if lnsb is None or ln_trivial[kind]:
                    return None, None
                if kind == 0:
                    gr, br = layer, L + layer
                elif kind == 1:
                    gr, br = 2 * L + layer, 3 * L + layer
                else:
                    gr, br = 4 * L, 4 * L + 1
                return lnsb[gr], lnsb[br]

            # ---- residual stream, channel-major fp32 ----
            xT = sb.tile([P, KO, TL], F32, tag="xT", bufs=1, name="xT")

            # ---- embedding: gather + pos add (token-major), then transpose --
            for tt in range(TT):
                idx_sb = sb.tile([P, 1], I32, tag="idx", bufs=2, name="idx_sb")
                nc.sync.dma_start(idx_sb[:], idx[tt * P:(tt + 1) * P, None])
                emb = sb.tile([P, D], BF16, tag="hT", bufs=2, name="emb")
                nc.gpsimd.indirect_dma_start(
                    out=emb[:], out_offset=None, in_=tok_emb[:, :],
                    in_offset=bass.IndirectOffsetOnAxis(ap=idx_sb[:, :1], axis=0),
                )
                pos_sb = sb.tile([P, D], F32, tag="ln_xbf", bufs=1, name="pos_sb")
                nc.sync.dma_start(pos_sb[:], pos[tt * P:(tt + 1) * P, :])
                x0 = sb.tile([P, D], F32, tag="ln_xsq", bufs=1, name="x0")
                nc.vector.tensor_tensor(x0[:], emb[:], pos_sb[:], op=ALU.add)
                for ko in range(KO):
                    pst = ps.tile([P, 512], F32, tag="ps", bufs=4, name="pst")
                    nc.tensor.transpose(
                        pst[:, :P], x0[:, ko * P:(ko + 1) * P], ident[:])
                    nc.vector.tensor_copy(
                        xT[:, ko, tt * P:(tt + 1) * P], pst[:, :P])

            # ================= transformer layers =================
            for l in range(L):
                # -- LN1 --
                xn1 = sb.tile([P, KO, TL], BF16, tag="xn", bufs=2, name="xn1")
                g, b = ln_gb(0, l)
                _ln_channel_major(nc, sb, ps, xT, ones_bf, eps_tile, xn1, g, b)

                # -- attention weights (bf16, lhsT layout [p, ko, m]) --
                wq_sb = sb.tile([P, KO, D], BF16, tag="wq", bufs=1, name="wq_sb")
                wk_sb = sb.tile([P, KO, D], BF16, tag="wk", bufs=1, name="wk_sb")
                wv_sb = sb.tile([P, KO, D], BF16, tag="wv", bufs=1, name="wv_sb")
                nc.sync.dma_start(
                    wq_sb[:], wq[l].rearrange("(ko p) m -> p ko m", p=P))
                nc.sync.dma_start(
                    wk_sb[:], wk[l].rearrange("(ko p) m -> p ko m", p=P))
                nc.sync.dma_start(
                    wv_sb[:], wv[l].rearrange("(ko p) m -> p ko m", p=P))

                # -- Q^T, K^T channel-major [P, KO, TL] bf16 --
                qT = sb.tile([P, KO, TL], BF16, tag="qT", bufs=1, name="qT")
                kT = sb.tile([P, KO, TL], BF16, tag="kT", bufs=1, name="kT")
                for w_sb, dst, scale in ((wq_sb, qT, 1.0 / np.sqrt(HD)),
                                         (wk_sb, kT, 1.0)):
                    for mp in range(KO // 2):
                        pq = ps.tile([P, 512], F32, tag="ps", bufs=4, name="pq")
                        for half in range(2):
                            m = 2 * mp + half
                            for ko in range(KO):
                                nc.tensor.matmul(
                                    pq[:, half * TL:half * TL + TL],
                                    lhsT=w_sb[:, ko, m * P:(m + 1) * P],
                                    rhs=xn1[:, ko, :],
                                    start=(ko == 0), stop=(ko == KO - 1))
                        nc.scalar.activation(
                            dst[:, 2 * mp:2 * mp + 2, :].rearrange(
                                "p a b -> p (a b)"),
                            pq[:], AF.Copy, scale=float(scale))

                # -- V token-major [P, TT, D] bf16 --
                vtok = sb.tile([P, TT, D], BF16, tag="vtok", bufs=1, name="vtok")
                for tt in range(TT):
                    for dh in range(D // 512):
                        pv = ps.tile([P, 512], F32, tag="ps", bufs=4, name="pv")
                        for ko in range(KO):
                            nc.tensor.matmul(
                                pv[:],
                                lhsT=xn1[:, ko, tt * P:(tt + 1) * P],
                                rhs=wv_sb[:, ko, dh * 512:(dh + 1) * 512],
                                start=(ko == 0), stop=(ko == KO - 1))
                        nc.vector.tensor_copy(
                            vtok[:, tt, dh * 512:(dh + 1) * 512], pv[:])

                # -- K/V all-gather within the 4-core group --
                kv_in = dram.tile([2, D * TL], BF16, tag="kv_in", bufs=2,
                                  name="kv_in")
                kv_out = dram.tile([GRP, 2, D * TL], BF16, tag="kv_out", bufs=2,
                                   name="kv_out")
                nc.sync.dma_start(
                    kv_in[0].rearrange("(ko p t) -> p ko t", p=P, t=TL), kT[:])
                nc.sync.dma_start(
                    kv_in[1].rearrange("(tt p d) -> p tt d", p=P, d=D), vtok[:])
                nc.gpsimd.collective_compute(
                    "AllGather", ALU.bypass, replica_groups=groups,
                    ins=[kv_in.opt()], outs=[kv_out.opt()])
                kg = sb.tile([P, GRP, KO, TL], BF16, tag="kg", bufs=1, name="kg")
                vg = sb.tile([P, GRP, TT, D], BF16, tag="vg", bufs=1, name="vg")
                for s in range(GRP):
                    nc.sync.dma_start(
                        kg[:, s],
                        kv_out[s, 0].rearrange("(ko p t) -> p ko t", p=P, t=TL))
                    nc.sync.dma_start(
                        vg[:, s],
                        kv_out[s, 1].rearrange("(tt p d) -> p tt d", p=P, d=D))

                # -- attention per head --
                yT = sb.tile([P, KO, TL], BF16, tag="yT", bufs=1, name="yT")
                for h in range(H):
                    hp = (h % 2) * HD
                    ko_h = h // 2
                    # S^T = K_h^T.T @ Q_h^T -> [keys, q], keys tiled (s, tt)
                    sps = [ps.tile([P, 512], F32, tag="ps", bufs=4,
                                   name=f"sps{i}") for i in range(4)]
                    for s in range(GRP):
                        for tt in range(TT):
                            kt = 2 * s + tt
                            nc.tensor.matmul(
                                sps[kt // 2][:, (kt % 2) * TL:(kt % 2) * TL + TL],
                                lhsT=kg[hp:hp + HD, s, ko_h, tt * P:(tt + 1) * P],
                                rhs=qT[hp:hp + HD, ko_h, :],
                                start=True, stop=True)
                    expS = sb.tile([P, 2 * GRP, TL], BF16, tag="expS", bufs=2,
                                   name="expS")
                    for i in range(4):
                        nc.scalar.activation(
                            expS[:, 2 * i:2 * i + 2, :].rearrange(
                                "p a b -> p (a b)"),
                            sps[i][:], AF.Exp)
                    nc.vector.tensor_tensor(
                        expS[:], expS[:], mask_sb[:], op=ALU.mult)
                    # Z (replicated over partitions) and 1/Z
                    zp = ps.tile([P, 512], F32, tag="ps", bufs=4, name="zp")
                    for kt in range(2 * GRP):
                        nc.tensor.matmul(
                            zp[:, :TL], lhsT=ones_bf[:], rhs=expS[:, kt, :],
                            start=(kt == 0), stop=(kt == 2 * GRP - 1))
                    rz = sb.tile([P, TL], F32, tag="rz", bufs=2, name="rz")
                    nc.vector.reciprocal(rz[:], zp[:, :TL])
                    # Y^T_h: lhsT = token-major V slice, rhs = expS^T
                    yp = ps.tile([P, 512], F32, tag="ps", bufs=4, name="yp")
                    for s in range(GRP):
                        for tt in range(TT):
                            kt = 2 * s + tt
                            nc.tensor.matmul(
                                yp[hp:hp + HD, :TL],
                                lhsT=vg[:, s, tt, h * HD:(h + 1) * HD],
                                rhs=expS[:, kt, :],
                                start=(kt == 0), stop=(kt == 2 * GRP - 1))
                    nc.vector.tensor_tensor(
                        yT[hp:hp + HD, ko_h, :], yp[hp:hp + HD, :TL],
                        rz[hp:hp + HD, :], op=ALU.mult)

                # -- out proj (reuses wv) + residual --
                for mp in range(KO // 2):
                    po = ps.tile([P, 512], F32, tag="ps", bufs=4, name="po")
                    for half in range(2):
                        m = 2 * mp + half
                        for ko in range(KO):
                            nc.tensor.matmul(
                                po[:, half * TL:half * TL + TL],
                                lhsT=wv_sb[:, ko, m * P:(m + 1) * P],
                                rhs=yT[:, ko, :],
                                start=(ko == 0), stop=(ko == KO - 1))
                    xflat = xT[:, 2 * mp:2 * mp + 2, :].rearrange(
                        "p a b -> p (a b)")
                    nc.vector.tensor_tensor(xflat, xflat, po[:], op=ALU.add)

                # -- LN2 --
                xn2 = sb.tile([P, KO, TL], BF16, tag="xn", bufs=2, name="xn2")
                g, b = ln_gb(1, l)
                _ln_channel_major(nc, sb, ps, xT, ones_bf, eps_tile, xn2, g, b)

                # -- MLP --
                # w1/gelu phase: materialize all of h (reuses the kg slot,
                # which is dead after the attention block)
                NCH = DH // 512
                h_full = sb.tile([P, DH // P, TL], BF16, tag="kg", bufs=1,
                                 name="h_full")
                for c in range(NCH):
                    w1c = sb.tile([P, KO, 512], BF16, tag="w1c", bufs=2,
                                  name="w1c")
                    nc.sync.dma_start(
                        w1c[:],
                        w1[l, :, c * 512:(c + 1) * 512].rearrange(
                            "(ko p) m -> p ko m", p=P))
                    for mp in range(2):
                        ph = ps.tile([P, 512], F32, tag="ps", bufs=4, name="ph")
                        for half in range(2):
                            m = 2 * mp + half
                            for ko in range(KO):
                                nc.tensor.matmul(
                                    ph[:, half * TL:half * TL + TL],
                                    lhsT=w1c[:, ko, m * P:(m + 1) * P],
                                    rhs=xn2[:, ko, :],
                                    start=(ko == 0), stop=(ko == KO - 1))
                        nc.scalar.activation(
                            h_full[:, c * 4 + 2 * mp:c * 4 + 2 * mp + 2, :]
                            .rearrange("p a b -> p (a b)"),
                            ph[:], GELU)
                # w2 phase: two passes over output-channel halves; each pass
                # runs 4 bank-exclusive accumulation chains over all of h
                for hp2 in range(2):
                    mo = [ps.tile([P, 512], F32, tag="ps", bufs=4,
                                  name=f"mo{i}") for i in range(4)]
                    for c in range(NCH):
                        w2c = sb.tile([P, 4, 512], BF16, tag="w2c", bufs=2,
                                      name="w2c")
                        nc.sync.dma_start(
                            w2c[:],
                            w2[l, c * 512:(c + 1) * 512,
                               hp2 * 512:(hp2 + 1) * 512].rearrange(
                                "(kc p) m -> p kc m", p=P))
                        for kc in range(4):
                            for m in range(4):
                                nc.tensor.matmul(
                                    mo[m][:, :TL],
                                    lhsT=w2c[:, kc, m * P:(m + 1) * P],
                                    rhs=h_full[:, c * 4 + kc, :],
                                    start=(c == 0 and kc == 0),
                                    stop=(c == NCH - 1 and kc == 3))
                    for m in range(4):
                        nc.vector.tensor_tensor(
                            xT[:, 4 * hp2 + m, :], xT[:, 4 * hp2 + m, :],
                            mo[m][:, :TL], op=ALU.add)

            # ================= final LN + LM head =================
            NVS = V // VS
            xnf = sb.tile([P, KO, TL], BF16, tag="xn", bufs=2, name="xnf")
            g, b = ln_gb(2, 0)
            _ln_channel_major(nc, sb, ps, xT, ones_bf, eps_tile, xnf, g, b)

            sumexp = sb.tile([P, TT], F32, tag="sumexp", bufs=1, name="sumexp")
            nc.vector.memset(sumexp[:], 0.0)
            for vs in range(NVS):
                lw = sb.tile([P, KO, VS], BF16, tag="lmw", bufs=2, name="lw")
                nc.sync.dma_start(
                    lw[:],
                    lm_w[:, vs * VS:(vs + 1) * VS].rearrange(
                        "(ko p) v -> p ko v", p=P))
                for tt in range(TT):
                    pl = ps.tile([P, 512], F32, tag="ps", bufs=4, name="pl")
                    for ko in range(KO):
                        nc.tensor.matmul(
                            pl[:, :VS], lhsT=xnf[:, ko, tt * P:(tt + 1) * P],
                            rhs=lw[:, ko, :],
                            start=(ko == 0), stop=(ko == KO - 1))
                    lsb = sb.tile([P, VS], F32, tag="lsb", bufs=2, name="lsb")
                    nc.vector.tensor_copy(lsb[:], pl[:, :VS])
                    nc.sync.dma_start(
                        logits[tt * P:(tt + 1) * P, vs * VS:(vs + 1) * VS],
                        lsb[:])
                    el = sb.tile([P, VS], F32, tag="el", bufs=2, name="el")
                    nc.scalar.activation(el[:], pl[:, :VS], AF.Exp)
                    se = sb.tile([P, 1], F32, tag="se", bufs=2, name="se")
                    nc.vector.tensor_reduce(se[:], el[:], axis=AX.X, op=ALU.add)
                    nc.vector.tensor_tensor(
                        sumexp[:, tt:tt + 1], sumexp[:, tt:tt + 1], se[:],
                        op=ALU.add)
            lse_sb = sb.tile([P, TT], F32, tag="lse", bufs=1, name="lse_sb")
            nc.scalar.activation(lse_sb[:], sumexp[:], AF.Ln)
            nc.sync.dma_start(lse[:, :], lse_sb[:])

    nc.compile()
    return nc


def _to_bf16(a):
    return np.ascontiguousarray(np.asarray(a).astype(ml_dtypes.bfloat16))


def make_in_maps(inp, ln_trivial):
    idx = np.asarray(inp["idx"]).astype(np.int32)
    pos_f = np.asarray(inp["pos_emb"]).astype(np.float32)
    tok_emb = _to_bf16(inp["tok_emb"])
    wq, wk, wv = _to_bf16(inp["wq"]), _to_bf16(inp["wk"]), _to_bf16(inp["wv"])
    w1, w2 = _to_bf16(inp["w1"]), _to_bf16(inp["w2"])
    lm_w = _to_bf16(inp["lm_w"])

    lnw = None
    if not all(ln_trivial):
        lnw = np.concatenate([
            np.asarray(inp["ln1_g"]), np.asarray(inp["ln1_b"]),
            np.asarray(inp["ln2_g"]), np.asarray(inp["ln2_b"]),
            np.asarray(inp["lnf_g"])[None], np.asarray(inp["lnf_b"])[None],
        ], axis=0).astype(np.float32)

    in_maps = []
    for c in range(NCORES):
        b, p = c // GRP, c % GRP
        tok0 = p * TL
        kp = np.arange(P)[:, None, None]
        kt = np.arange(2 * GRP)[None, :, None]
        q = np.arange(TL)[None, None, :]
        mask = ((kt * P + kp) <= (tok0 + q)).astype(ml_dtypes.bfloat16)
        m = {
            "tok_emb": tok_emb,
            "idx": np.ascontiguousarray(idx[b, tok0:tok0 + TL]),
            "pos": np.ascontiguousarray(pos_f[tok0:tok0 + TL]),
            "maskT": np.ascontiguousarray(mask),
            "wq": wq, "wk": wk, "wv": wv, "w1": w1, "w2": w2,
            "lm_w": lm_w,
        }
        if lnw is not None:
            m["lnw"] = lnw
        in_maps.append(m)
    return in_maps


def assemble(results, targets):
    logits = np.concatenate(
        [results[c]["logits"] for c in range(NCORES)], axis=0)
    lse = np.concatenate(
        [results[c]["lse"].T.reshape(-1) for c in range(NCORES)], axis=0)
    tgt = np.asarray(targets).reshape(-1).astype(np.int64)
    nll = lse.astype(np.float64) - logits[np.arange(len(tgt)), tgt].astype(np.float64)
    loss = np.float32(np.mean(nll))
    return logits, loss


def kernel(**inputs):
    inp = {k: np.asarray(v) for k, v in inputs.items()}
    for name in ("bq", "bk", "bv", "b1", "b2", "lm_b"):
        assert not np.any(inp[name]), f"{name} must be zero (unsupported)"

    ln_trivial = (
        bool(np.all(inp["ln1_g"] == 1) and not np.any(inp["ln1_b"])),
        bool(np.all(inp["ln2_g"] == 1) and not np.any(inp["ln2_b"])),
        bool(np.all(inp["lnf_g"] == 1) and not np.any(inp["lnf_b"])),
    )

    key = ("gpt", ln_trivial)
    if key not in _CACHED:
        _CACHED[key] = build_nc(ln_trivial)
    nc = _CACHED[key]

    in_maps = make_in_maps(inp, ln_trivial)
    res = run_bass_kernel_spmd(nc, in_maps, core_ids=list(range(NCORES)))
    return assemble(res.results, inp["targets"])


if __name__ == "__main__":
    build_nc((True, True, True))
    print("built ok")


# revision 11
# speedup vs baseline: 1.1778x; 1.1778x over previous
"""Trainium2 Bass kernel for a 6-layer GPT forward pass (logits + CE loss).

Sharding: 8-way sequence-parallel. B=2 batch rows x 4 chunks of 256 tokens.
Core c handles batch row c//4, token chunk c%4. Per layer, K/V are
all-gathered within each 4-core group; everything else is token-local.
The LM head + per-token sum-exp are computed locally per core; the host
assembles the full logits and the scalar loss.

Activations live channel-major ("x^T": [128 part, D/128, T_local]) in SBUF
for the whole trunk; matmul weights stream from HBM as bf16.
"""

import sys

for _p in ("/opt/trn_rl_repo", "/root/.axon_site/_ro/trn_rl_repo"):
    if _p not in sys.path:
        sys.path.insert(0, _p)

import numpy as np
import ml_dtypes

import concourse.bass as bass
import concourse.mybir as mybir
import concourse.tile as tile
from concourse import bacc
from concourse.bass_utils import run_bass_kernel_spmd
from concourse.masks import make_identity

F32 = mybir.dt.float32
BF16 = mybir.dt.bfloat16
I32 = mybir.dt.int32
AF = mybir.ActivationFunctionType
ALU = mybir.AluOpType
AX = mybir.AxisListType

# model dims (hardcoded per problem spec); V/L/VS are module globals so a
# small-config simulator harness can shrink them before building.
V, D, H, L, T, B = 32000, 1024, 16, 6, 1024, 2
VS = 500                 # vocab slice width for the LM head (V % VS == 0)
HD = D // H              # 64
P = 128
KO = D // P              # 8  (channel tiles)
NCORES = 8
GRP = 4                  # cores per batch row
TL = T // GRP            # 256 tokens per core
TT = TL // P             # 2  (token tiles)
DH = 4 * D               # 4096
EPS = 1e-5
GELU = AF.Gelu   # dev_sim swaps this for a sim-supported function

_CACHED = {}


def _ln_channel_major(nc, sb, ps, xT, ones_bf, eps_tile, out_bf, g_tile, b_tile):
    """LayerNorm over channels for channel-major xT [P, KO, TL] (fp32).

    Stats are computed replicated across partitions via ones-matmuls on a
    bf16 copy of x.  Writes normalized bf16 to out_bf [P, KO, TL].
    g_tile/b_tile: optional [P, KO] fp32 per-channel gain/bias.
    """
    x_bf = sb.tile([P, KO, TL], BF16, tag="ln_xbf", bufs=1, name="ln_xbf")
    nc.vector.tensor_copy(x_bf[:], xT[:])
    xsq_bf = sb.tile([P, KO, TL], BF16, tag="ln_xsq", bufs=1, name="ln_xsq")
    nc.scalar.square(xsq_bf[:], xT[:])

    sums = ps.tile([P, 512], F32, tag="ps", bufs=8, name="ln_sums")
    sumsq = ps.tile([P, 512], F32, tag="ps", bufs=8, name="ln_sumsq")
    for ko in range(KO):
        nc.tensor.matmul(sums[:, :TL], lhsT=ones_bf[:], rhs=x_bf[:, ko, :],
                         start=(ko == 0), stop=(ko == KO - 1))
    for ko in range(KO):
        nc.tensor.matmul(sumsq[:, :TL], lhsT=ones_bf[:], rhs=xsq_bf[:, ko, :],
                         start=(ko == 0), stop=(ko == KO - 1))

    mu = sb.tile([P, TL], F32, tag="ln_mu", bufs=2, name="ln_mu")
    nc.scalar.mul(mu[:], sums[:, :TL], 1.0 / D)
    # var = sumsq/D - mu^2 ; rstd = 1/sqrt(var+eps)
    var = sb.tile([P, TL], F32, tag="ln_var", bufs=2, name="ln_var")
    nc.vector.tensor_tensor(var[:], mu[:], mu[:], op=ALU.mult)
    nc.vector.scalar_tensor_tensor(
        out=var[:], in0=sumsq[:, :TL], scalar=1.0 / D, in1=var[:],
        op0=ALU.mult, op1=ALU.subtract)
    rstd = sb.tile([P, TL], F32, tag="ln_rstd", bufs=2, name="ln_rstd")
    nc.scalar.activation(rstd[:], var[:], AF.Sqrt, bias=eps_tile[:])
    nc.vector.reciprocal(rstd[:], rstd[:])

    xc = x_bf  # reuse: sums matmuls are the last reader of x_bf
    nc.vector.tensor_tensor(
        xc[:], xT[:], mu[:, None, :].to_broadcast([P, KO, TL]), op=ALU.subtract)
    if g_tile is not None:
        nc.vector.tensor_tensor(
            xc[:], xc[:], g_tile[:, :, None].to_broadcast([P, KO, TL]),
            op=ALU.mult)
    nc.vector.tensor_tensor(
        out_bf[:], xc[:], rstd[:, None, :].to_broadcast([P, KO, TL]),
        op=ALU.mult)
    if b_tile is not None:
        nc.vector.tensor_tensor(
            out_bf[:], out_bf[:], b_tile[:, :, None].to_broadcast([P, KO, TL]),
            op=ALU.add)


def build_nc(ln_trivial):
    """Build the SPMD program. ln_trivial: (ln1, ln2, lnf) flags for
    all-ones gain / zero bias, decided from actual inputs at build time."""
    NVS = V // VS
    assert V % VS == 0

    nc = bacc.Bacc("TRN2", target_bir_lowering=False, debug=False,
                   num_devices=NCORES)

    # ---- per-core DRAM inputs ----
    tok_emb = nc.dram_tensor("tok_emb", [V, D], BF16, kind="ExternalInput")
    idx = nc.dram_tensor("idx", [TL], I32, kind="ExternalInput")
    pos = nc.dram_tensor("pos", [TL, D], F32, kind="ExternalInput")
    maskT = nc.dram_tensor("maskT", [P, 2 * GRP, TL], BF16, kind="ExternalInput")
    wq = nc.dram_tensor("wq", [L, D, D], BF16, kind="ExternalInput")
    wk = nc.dram_tensor("wk", [L, D, D], BF16, kind="ExternalInput")
    wv = nc.dram_tensor("wv", [L, D, D], BF16, kind="ExternalInput")
    w1 = nc.dram_tensor("w1", [L, D, DH], BF16, kind="ExternalInput")
    w2 = nc.dram_tensor("w2", [L, DH, D], BF16, kind="ExternalInput")
    lnw = None
    if not all(ln_trivial):
        # rows: ln1_g(L), ln1_b(L), ln2_g(L), ln2_b(L), lnf_g, lnf_b
        lnw = nc.dram_tensor("lnw", [4 * L + 2, D], F32, kind="ExternalInput")
    lm_w = nc.dram_tensor("lm_w", [D, V], BF16, kind="ExternalInput")

    # ---- per-core DRAM outputs ----
    logits = nc.dram_tensor("logits", [TL, V], F32, kind="ExternalOutput")
    lse = nc.dram_tensor("lse", [P, TT], F32, kind="ExternalOutput")

    groups = [[0, 1, 2, 3], [4, 5, 6, 7]]

    with tile.TileContext(nc) as tc:
        with (
            tc.tile_pool(name="sb", bufs=2) as sb,
            tc.tile_pool(name="ps", bufs=8, space="PSUM") as ps,
            tc.tile_pool(name="dram", bufs=2, space="DRAM") as dram,
        ):
            ones_bf = sb.tile([P, P], BF16, tag="ones", bufs=1, name="ones")
            nc.vector.memset(ones_bf[:], 1.0)
            ident = sb.tile([P, P], F32, tag="ident", bufs=1, name="ident")
            make_identity(nc, ident[:])
            mask_sb = sb.tile([P, 2 * GRP, TL], BF16, tag="mask", bufs=1,
                              name="mask_sb")
            eps_tile = sb.tile([P, 1], F32, tag="eps", bufs=1, name="eps_tile")
            nc.vector.memset(eps_tile[:], EPS)
            nc.sync.dma_start(mask_sb[:], maskT[:, :, :])

            lnsb = None
            if lnw is not None:
                lnsb = sb.tile([4 * L + 2, P, KO], F32, tag="lnsb", bufs=1,
                               name="lnsb")
                nc.sync.dma_start(
                    lnsb[:], lnw[:, :].rearrange("n (ko p) -> n p ko", p=P))

            def ln_gb(kind, layer):
                if lnsb is None or ln_trivial[kind]:
                    return None, None
                if kind == 0:
                    gr, br = layer, L + layer
                elif kind == 1:
                    gr, br = 2 * L + layer, 3 * L + layer
                else:
                    gr, br = 4 * L, 4 * L + 1
                return lnsb[gr], lnsb[br]

            # ---- residual stream, channel-major fp32 ----
            xT = sb.tile([P, KO, TL], F32, tag="xT", bufs=1, name="xT")

            # ---- embedding: gather + pos add (token-major), then transpose --
            for tt in range(TT):
                idx_sb = sb.tile([P, 1], I32, tag="idx", bufs=2, name="idx_sb")
                nc.sync.dma_start(idx_sb[:], idx[tt * P:(tt + 1) * P, None])
                emb = sb.tile([P, D], BF16, tag="hT", bufs=2, name="emb")
                nc.gpsimd.indirect_dma_start(
                    out=emb[:], out_offset=None, in_=tok_emb[:, :],
                    in_offset=bass.IndirectOffsetOnAxis(ap=idx_sb[:, :1], axis=0),
                )
                pos_sb = sb.tile([P, D], F32, tag="ln_xbf", bufs=1, name="pos_sb")
                nc.sync.dma_start(pos_sb[:], pos[tt * P:(tt + 1) * P, :])
                x0 = sb.tile([P, D], F32, tag="ln_xsq", bufs=1, name="x0")
                nc.vector.tensor_tensor(x0[:], emb[:], pos_sb[:], op=ALU.add)
                for ko in range(KO):
                    pst = ps.tile([P, 512], F32, tag="ps", bufs=8, name="pst")
                    nc.tensor.transpose(
                        pst[:, :P], x0[:, ko * P:(ko + 1) * P], ident[:])
                    nc.vector.tensor_copy(
                        xT[:, ko, tt * P:(tt + 1) * P], pst[:, :P])

            # ================= transformer layers =================
            for l in range(L):
                # -- LN1 --
                xn1 = sb.tile([P, KO, TL], BF16, tag="xn", bufs=2, name="xn1")
                g, b = ln_gb(0, l)
                _ln_channel_major(nc, sb, ps, xT, ones_bf, eps_tile, xn1, g, b)

                # -- attention weights (bf16, lhsT layout [p, ko, m]) --
                wq_sb = sb.tile([P, KO, D], BF16, tag="wq", bufs=1, name="wq_sb")
                wk_sb = sb.tile([P, KO, D], BF16, tag="wk", bufs=1, name="wk_sb")
                wv_sb = sb.tile([P, KO, D], BF16, tag="wv", bufs=1, name="wv_sb")
                nc.sync.dma_start(
                    wq_sb[:], wq[l].rearrange("(ko p) m -> p ko m", p=P))
                nc.sync.dma_start(
                    wk_sb[:], wk[l].rearrange("(ko p) m -> p ko m", p=P))
                nc.sync.dma_start(
                    wv_sb[:], wv[l].rearrange("(ko p) m -> p ko m", p=P))

                # -- K^T channel-major, V token-major; gather early so the
                # collective overlaps the Q projection --
                qT = sb.tile([P, KO, TL], BF16, tag="qT", bufs=1, name="qT")
                kT = sb.tile([P, KO, TL], BF16, tag="kT", bufs=1, name="kT")
                for mp in range(KO // 2):
                    pq = ps.tile([P, 512], F32, tag="ps", bufs=8, name="pq")
                    for half in range(2):
                        m = 2 * mp + half
                        for ko in range(KO):
                            nc.tensor.matmul(
                                pq[:, half * TL:half * TL + TL],
                                lhsT=wk_sb[:, ko, m * P:(m + 1) * P],
                                rhs=xn1[:, ko, :],
                                start=(ko == 0), stop=(ko == KO - 1))
                    nc.scalar.activation(
                        kT[:, 2 * mp:2 * mp + 2, :].rearrange(
                            "p a b -> p (a b)"),
                        pq[:], AF.Copy)

                vtok = sb.tile([P, TT, D], BF16, tag="vtok", bufs=1, name="vtok")
                for tt in range(TT):
                    for dh in range(D // 512):
                        pv = ps.tile([P, 512], F32, tag="ps", bufs=8, name="pv")
                        for ko in range(KO):
                            nc.tensor.matmul(
                                pv[:],
                                lhsT=xn1[:, ko, tt * P:(tt + 1) * P],
                                rhs=wv_sb[:, ko, dh * 512:(dh + 1) * 512],
                                start=(ko == 0), stop=(ko == KO - 1))
                        nc.vector.tensor_copy(
                            vtok[:, tt, dh * 512:(dh + 1) * 512], pv[:])

                # -- K/V all-gather within the 4-core group --
                kv_in = dram.tile([2, D * TL], BF16, tag="kv_in", bufs=2,
                                  name="kv_in")
                kv_out = dram.tile([GRP, 2, D * TL], BF16, tag="kv_out", bufs=2,
                                   name="kv_out")
                nc.sync.dma_start(
                    kv_in[0].rearrange("(ko p t) -> p ko t", p=P, t=TL), kT[:])
                nc.sync.dma_start(
                    kv_in[1].rearrange("(tt p d) -> p tt d", p=P, d=D), vtok[:])
                nc.gpsimd.collective_compute(
                    "AllGather", ALU.bypass, replica_groups=groups,
                    ins=[kv_in.opt()], outs=[kv_out.opt()])
                kg = sb.tile([P, GRP, KO, TL], BF16, tag="kg", bufs=1, name="kg")
                vg = sb.tile([P, GRP, TT, D], BF16, tag="vg", bufs=1, name="vg")
                for s in range(GRP):
                    nc.sync.dma_start(
                        kg[:, s],
                        kv_out[s, 0].rearrange("(ko p t) -> p ko t", p=P, t=TL))
                    nc.sync.dma_start(
                        vg[:, s],
                        kv_out[s, 1].rearrange("(tt p d) -> p tt d", p=P, d=D))

                # -- Q^T (overlaps the gather) --
                for mp in range(KO // 2):
                    pq = ps.tile([P, 512], F32, tag="ps", bufs=8, name="pq")
                    for half in range(2):
                        m = 2 * mp + half
                        for ko in range(KO):
                            nc.tensor.matmul(
                                pq[:, half * TL:half * TL + TL],
                                lhsT=wq_sb[:, ko, m * P:(m + 1) * P],
                                rhs=xn1[:, ko, :],
                                start=(ko == 0), stop=(ko == KO - 1))
                    nc.scalar.activation(
                        qT[:, 2 * mp:2 * mp + 2, :].rearrange(
                            "p a b -> p (a b)"),
                        pq[:], AF.Copy, scale=float(1.0 / np.sqrt(HD)))

                # -- attention, one head pair at a time --
                yT = sb.tile([P, KO, TL], BF16, tag="yT", bufs=1, name="yT")
                for j in range(H // 2):          # heads a=2j (parts 0:64), b=2j+1
                    sps = {}
                    for hb in range(2):          # head in pair
                        hp = hb * HD
                        for i in range(4):
                            sps[hb, i] = ps.tile([P, 512], F32, tag="ps",
                                                 bufs=8, name=f"sps{hb}{i}")
                    for s in range(GRP):
                        for tt in range(TT):
                            kt = 2 * s + tt
                            for hb in range(2):  # adjacent row-groups overlap
                                hp = hb * HD
                                nc.tensor.matmul(
                                    sps[hb, kt // 2][:, (kt % 2) * TL:
                                                     (kt % 2) * TL + TL],
                                    lhsT=kg[hp:hp + HD, s, j,
                                            tt * P:(tt + 1) * P],
                                    rhs=qT[hp:hp + HD, j, :],
                                    start=True, stop=True)
                    expS = {}
                    for hb in range(2):
                        expS[hb] = sb.tile([P, 2 * GRP, TL], BF16, tag="expS",
                                           bufs=3, name=f"expS{hb}")
                        for i in range(4):
                            nc.scalar.activation(
                                expS[hb][:, 2 * i:2 * i + 2, :].rearrange(
                                    "p a b -> p (a b)"),
                                sps[hb, i][:], AF.Exp)
                        nc.vector.tensor_tensor(
                            expS[hb][:], expS[hb][:], mask_sb[:], op=ALU.mult)
                    # Z/Y for both heads packed into shared banks via
                    # sequential per-head chains on disjoint col groups
                    zp = ps.tile([P, 512], F32, tag="ps", bufs=8, name="zp")
                    for hb in range(2):
                        hp = hb * HD
                        for kt in range(2 * GRP):
                            nc.tensor.matmul(
                                zp[hp:hp + HD, :TL], lhsT=ones_bf[:, :HD],
                                rhs=expS[hb][:, kt, :],
                                start=(kt == 0), stop=(kt == 2 * GRP - 1))
                    yp = ps.tile([P, 512], F32, tag="ps", bufs=8, name="yp")
                    for hb in range(2):
                        hp = hb * HD
                        h = 2 * j + hb
                        for s in range(GRP):
                            for tt in range(TT):
                                kt = 2 * s + tt
                                nc.tensor.matmul(
                                    yp[hp:hp + HD, :TL],
                                    lhsT=vg[:, s, tt, h * HD:(h + 1) * HD],
                                    rhs=expS[hb][:, kt, :],
                                    start=(kt == 0), stop=(kt == 2 * GRP - 1))
                    rz = sb.tile([P, TL], F32, tag="rz", bufs=2, name="rz")
                    nc.vector.reciprocal_approx_fast(rz[:], zp[:, :TL])
                    nc.vector.tensor_tensor(
                        yT[:, j, :], yp[:, :TL], rz[:], op=ALU.mult)

                # -- out proj (reuses wv) + residual --
                for mp in range(KO // 2):
                    po = ps.tile([P, 512], F32, tag="ps", bufs=8, name="po")
                    for half in range(2):
                        m = 2 * mp + half
                        for ko in range(KO):
                            nc.tensor.matmul(
                                po[:, half * TL:half * TL + TL],
                                lhsT=wv_sb[:, ko, m * P:(m + 1) * P],
                                rhs=yT[:, ko, :],
                                start=(ko == 0), stop=(ko == KO - 1))
                    xflat = xT[:, 2 * mp:2 * mp + 2, :].rearrange(
                        "p a b -> p (a b)")
                    nc.vector.tensor_tensor(xflat, xflat, po[:], op=ALU.add)

                # -- LN2 --
                xn2 = sb.tile([P, KO, TL], BF16, tag="xn", bufs=2, name="xn2")
                g, b = ln_gb(1, l)
                _ln_channel_major(nc, sb, ps, xT, ones_bf, eps_tile, xn2, g, b)

                # -- MLP --
                # w1/gelu phase: materialize all of h (reuses the kg slot,
                # which is dead after the attention block)
                NCH = DH // 512
                h_full = sb.tile([P, DH // P, TL], BF16, tag="kg", bufs=1,
                                 name="h_full")
                for c in range(NCH):
                    w1c = sb.tile([P, KO, 512], BF16, tag="w1c", bufs=2,
                                  name="w1c")
                    nc.sync.dma_start(
                        w1c[:],
                        w1[l, :, c * 512:(c + 1) * 512].rearrange(
                            "(ko p) m -> p ko m", p=P))
                    for mp in range(2):
                        ph = ps.tile([P, 512], F32, tag="ps", bufs=8, name="ph")
                        for half in range(2):
                            m = 2 * mp + half
                            for ko in range(KO):
                                nc.tensor.matmul(
                                    ph[:, half * TL:half * TL + TL],
                                    lhsT=w1c[:, ko, m * P:(m + 1) * P],
                                    rhs=xn2[:, ko, :],
                                    start=(ko == 0), stop=(ko == KO - 1))
                        nc.scalar.activation(
                            h_full[:, c * 4 + 2 * mp:c * 4 + 2 * mp + 2, :]
                            .rearrange("p a b -> p (a b)"),
                            ph[:], GELU)
                # w2 phase: two passes over output-channel halves; each pass
                # runs 4 bank-exclusive accumulation chains over all of h
                for hp2 in range(2):
                    mo = [ps.tile([P, 512], F32, tag="ps", bufs=8,
                                  name=f"mo{i}") for i in range(4)]
                    for c in range(NCH):
                        w2c = sb.tile([P, 4, 512], BF16, tag="w2c", bufs=2,
                                      name="w2c")
                        nc.sync.dma_start(
                            w2c[:],
                            w2[l, c * 512:(c + 1) * 512,
                               hp2 * 512:(hp2 + 1) * 512].rearrange(
                                "(kc p) m -> p kc m", p=P))
                        for kc in range(4):
                            for m in range(4):
                                nc.tensor.matmul(
                                    mo[m][:, :TL],
                                    lhsT=w2c[:, kc, m * P:(m + 1) * P],
                                    rhs=h_full[:, c * 4 + kc, :],
                                    start=(c == 0 and kc == 0),
                                    stop=(c == NCH - 1 and kc == 3))
                    for m in range(4):
                        nc.vector.tensor_tensor(
                            xT[:, 4 * hp2 + m, :], xT[:, 4 * hp2 + m, :],
                            mo[m][:, :TL], op=ALU.add)

            # ================= final LN + LM head =================
            NVS = V // VS
            xnf = sb.tile([P, KO, TL], BF16, tag="xn", bufs=2, name="xnf")
            g, b = ln_gb(2, 0)
            _ln_channel_major(nc, sb, ps, xT, ones_bf, eps_tile, xnf, g, b)

            sumexp = sb.tile([P, TT], F32, tag="sumexp", bufs=1, name="sumexp")
            nc.vector.memset(sumexp[:], 0.0)
            for vs in range(NVS):
                lw = sb.tile([P, KO, VS], BF16, tag="lmw", bufs=3, name="lw")
                nc.sync.dma_start(
                    lw[:],
                    lm_w[:, vs * VS:(vs + 1) * VS].rearrange(
                        "(ko p) v -> p ko v", p=P))
                for tt in range(TT):
                    pl = ps.tile([P, 512], F32, tag="ps", bufs=8, name="pl")
                    for ko in range(KO):
                        nc.tensor.matmul(
                            pl[:, :VS], lhsT=xnf[:, ko, tt * P:(tt + 1) * P],
                            rhs=lw[:, ko, :],
                            start=(ko == 0), stop=(ko == KO - 1))
                    lsb = sb.tile([P, VS], F32, tag="lsb", bufs=2, name="lsb")
                    nc.vector.tensor_copy(lsb[:], pl[:, :VS])
                    nc.sync.dma_start(
                        logits[tt * P:(tt + 1) * P, vs * VS:(vs + 1) * VS],
                        lsb[:])
                    el = sb.tile([P, VS], F32, tag="el", bufs=2, name="el")
                    nc.scalar.activation(el[:], pl[:, :VS], AF.Exp)
                    se = sb.tile([P, 1], F32, tag="se", bufs=2, name="se")
                    nc.vector.tensor_reduce(se[:], el[:], axis=AX.X, op=ALU.add)
                    nc.vector.tensor_tensor(
                        sumexp[:, tt:tt + 1], sumexp[:, tt:tt + 1], se[:],
                        op=ALU.add)
            lse_sb = sb.tile([P, TT], F32, tag="lse", bufs=1, name="lse_sb")
            nc.scalar.activation(lse_sb[:], sumexp[:], AF.Ln)
            nc.sync.dma_start(lse[:, :], lse_sb[:])

    nc.compile()
    return nc


def _to_bf16(a):
    return np.ascontiguousarray(np.asarray(a).astype(ml_dtypes.bfloat16))


def make_in_maps(inp, ln_trivial):
    idx = np.asarray(inp["idx"]).astype(np.int32)
    pos_f = np.asarray(inp["pos_emb"]).astype(np.float32)
    tok_emb = _to_bf16(inp["tok_emb"])
    wq, wk, wv = _to_bf16(inp["wq"]), _to_bf16(inp["wk"]), _to_bf16(inp["wv"])
    w1, w2 = _to_bf16(inp["w1"]), _to_bf16(inp["w2"])
    lm_w = _to_bf16(inp["lm_w"])

    lnw = None
    if not all(ln_trivial):
        lnw = np.concatenate([
            np.asarray(inp["ln1_g"]), np.asarray(inp["ln1_b"]),
            np.asarray(inp["ln2_g"]), np.asarray(inp["ln2_b"]),
            np.asarray(inp["lnf_g"])[None], np.asarray(inp["lnf_b"])[None],
        ], axis=0).astype(np.float32)

    in_maps = []
    for c in range(NCORES):
        b, p = c // GRP, c % GRP
        tok0 = p * TL
        kp = np.arange(P)[:, None, None]
        kt = np.arange(2 * GRP)[None, :, None]
        q = np.arange(TL)[None, None, :]
        mask = ((kt * P + kp) <= (tok0 + q)).astype(ml_dtypes.bfloat16)
        m = {
            "tok_emb": tok_emb,
            "idx": np.ascontiguousarray(idx[b, tok0:tok0 + TL]),
            "pos": np.ascontiguousarray(pos_f[tok0:tok0 + TL]),
            "maskT": np.ascontiguousarray(mask),
            "wq": wq, "wk": wk, "wv": wv, "w1": w1, "w2": w2,
            "lm_w": lm_w,
        }
        if lnw is not None:
            m["lnw"] = lnw
        in_maps.append(m)
    return in_maps


def assemble(results, targets):
    logits = np.concatenate(
        [results[c]["logits"] for c in range(NCORES)], axis=0)
    lse = np.concatenate(
        [results[c]["lse"].T.reshape(-1) for c in range(NCORES)], axis=0)
    tgt = np.asarray(targets).reshape(-1).astype(np.int64)
    nll = lse.astype(np.float64) - logits[np.arange(len(tgt)), tgt].astype(np.float64)
    loss = np.float32(np.mean(nll))
    return logits, loss


def kernel(**inputs):
    inp = {k: np.asarray(v) for k, v in inputs.items()}
    for name in ("bq", "bk", "bv", "b1", "b2", "lm_b"):
        assert not np.any(inp[name]), f"{name} must be zero (unsupported)"

    ln_trivial = (
        bool(np.all(inp["ln1_g"] == 1) and not np.any(inp["ln1_b"])),
        bool(np.all(inp["ln2_g"] == 1) and not np.any(inp["ln2_b"])),
        bool(np.all(inp["lnf_g"] == 1) and not np.any(inp["lnf_b"])),
    )

    key = ("gpt", ln_trivial)
    if key not in _CACHED:
        _CACHED[key] = build_nc(ln_trivial)
    nc = _CACHED[key]

    in_maps = make_in_maps(inp, ln_trivial)
    res = run_bass_kernel_spmd(nc, in_maps, core_ids=list(range(NCORES)))
    return assemble(res.results, inp["targets"])


if __name__ == "__main__":
    build_nc((True, True, True))
    print("built ok")


# revision 12
# speedup vs baseline: 1.3041x; 1.1073x over previous
"""Trainium2 Bass kernel for a 6-layer GPT forward pass (logits + CE loss).

Sharding: 8-way sequence-parallel. B=2 batch rows x 4 chunks of 256 tokens.
Core c handles batch row c//4, token chunk c%4. Per layer, K/V are
all-gathered within each 4-core group; everything else is token-local.
The LM head + per-token sum-exp are computed locally per core; the host
assembles the full logits and the scalar loss.

Activations live channel-major ("x^T": [128 part, D/128, T_local]) in SBUF
for the whole trunk; matmul weights stream from HBM as bf16.
"""

import sys

for _p in ("/opt/trn_rl_repo", "/root/.axon_site/_ro/trn_rl_repo"):
    if _p not in sys.path:
        sys.path.insert(0, _p)

import numpy as np
import ml_dtypes

import concourse.bass as bass
import concourse.mybir as mybir
import concourse.tile as tile
from concourse import bacc
from concourse.bass_utils import run_bass_kernel_spmd
from concourse.masks import make_identity

F32 = mybir.dt.float32
BF16 = mybir.dt.bfloat16
I32 = mybir.dt.int32
AF = mybir.ActivationFunctionType
ALU = mybir.AluOpType
AX = mybir.AxisListType

# model dims (hardcoded per problem spec); V/L/VS are module globals so a
# small-config simulator harness can shrink them before building.
V, D, H, L, T, B = 32000, 1024, 16, 6, 1024, 2
VS = 500                 # vocab slice width for the LM head (V % VS == 0)
HD = D // H              # 64
P = 128
KO = D // P              # 8  (channel tiles)
NCORES = 8
GRP = 4                  # cores per batch row
TL = T // GRP            # 256 tokens per core
TT = TL // P             # 2  (token tiles)
DH = 4 * D               # 4096
EPS = 1e-5
GELU = AF.Gelu   # dev_sim swaps this for a sim-supported function

_CACHED = {}


def _ln_channel_major(nc, sb, ps, xT, ones_bf, eps_tile, out_bf, g_tile, b_tile):
    """LayerNorm over channels for channel-major xT [P, KO, TL] (fp32).

    Stats are computed replicated across partitions via ones-matmuls on a
    bf16 copy of x.  Writes normalized bf16 to out_bf [P, KO, TL].
    g_tile/b_tile: optional [P, KO] fp32 per-channel gain/bias.
    """
    x_bf = sb.tile([P, KO, TL], BF16, tag="ln_xbf", bufs=1, name="ln_xbf")
    nc.vector.tensor_copy(x_bf[:], xT[:])
    xsq_bf = sb.tile([P, KO, TL], BF16, tag="ln_xsq", bufs=1, name="ln_xsq")
    nc.scalar.square(xsq_bf[:], xT[:])

    sums = ps.tile([P, 512], F32, tag="ps", bufs=8, name="ln_sums")
    sumsq = ps.tile([P, 512], F32, tag="ps", bufs=8, name="ln_sumsq")
    for ko in range(KO):
        nc.tensor.matmul(sums[:, :TL], lhsT=ones_bf[:], rhs=x_bf[:, ko, :],
                         start=(ko == 0), stop=(ko == KO - 1))
    for ko in range(KO):
        nc.tensor.matmul(sumsq[:, :TL], lhsT=ones_bf[:], rhs=xsq_bf[:, ko, :],
                         start=(ko == 0), stop=(ko == KO - 1))

    mu = sb.tile([P, TL], F32, tag="ln_mu", bufs=2, name="ln_mu")
    nc.scalar.mul(mu[:], sums[:, :TL], 1.0 / D)
    # var = sumsq/D - mu^2 ; rstd = 1/sqrt(var+eps)
    var = sb.tile([P, TL], F32, tag="ln_var", bufs=2, name="ln_var")
    nc.vector.tensor_tensor(var[:], mu[:], mu[:], op=ALU.mult)
    nc.vector.scalar_tensor_tensor(
        out=var[:], in0=sumsq[:, :TL], scalar=1.0 / D, in1=var[:],
        op0=ALU.mult, op1=ALU.subtract)
    rstd = sb.tile([P, TL], F32, tag="ln_rstd", bufs=2, name="ln_rstd")
    nc.scalar.activation(rstd[:], var[:], AF.Sqrt, bias=eps_tile[:])
    nc.vector.reciprocal(rstd[:], rstd[:])

    xc = x_bf  # reuse: sums matmuls are the last reader of x_bf
    nc.vector.tensor_tensor(
        xc[:], xT[:], mu[:, None, :].to_broadcast([P, KO, TL]), op=ALU.subtract)
    if g_tile is not None:
        nc.vector.tensor_tensor(
            xc[:], xc[:], g_tile[:, :, None].to_broadcast([P, KO, TL]),
            op=ALU.mult)
    nc.vector.tensor_tensor(
        out_bf[:], xc[:], rstd[:, None, :].to_broadcast([P, KO, TL]),
        op=ALU.mult)
    if b_tile is not None:
        nc.vector.tensor_tensor(
            out_bf[:], out_bf[:], b_tile[:, :, None].to_broadcast([P, KO, TL]),
            op=ALU.add)


def build_nc(ln_trivial):
    """Build the SPMD program. ln_trivial: (ln1, ln2, lnf) flags for
    all-ones gain / zero bias, decided from actual inputs at build time."""
    NVS = V // VS
    assert V % VS == 0

    nc = bacc.Bacc("TRN2", target_bir_lowering=False, debug=False,
                   num_devices=NCORES)

    # ---- per-core DRAM inputs ----
    tok_emb = nc.dram_tensor("tok_emb", [V, D], BF16, kind="ExternalInput")
    idx = nc.dram_tensor("idx", [TL], I32, kind="ExternalInput")
    pos = nc.dram_tensor("pos", [TL, D], F32, kind="ExternalInput")
    maskT = nc.dram_tensor("maskT", [P, 2 * GRP, TL], BF16, kind="ExternalInput")
    # weights arrive pre-laid-out on the host so every DMA is one
    # contiguous segment per partition
    wq = nc.dram_tensor("wq", [L, P, KO, D], BF16, kind="ExternalInput")
    wk = nc.dram_tensor("wk", [L, P, KO, D], BF16, kind="ExternalInput")
    wv = nc.dram_tensor("wv", [L, P, KO, D], BF16, kind="ExternalInput")
    w1 = nc.dram_tensor("w1", [L, DH // 512, P, KO, 512], BF16,
                        kind="ExternalInput")
    w2 = nc.dram_tensor("w2", [L, 2, DH // 512, P, 4, 512], BF16,
                        kind="ExternalInput")
    lnw = None
    if not all(ln_trivial):
        # rows: ln1_g(L), ln1_b(L), ln2_g(L), ln2_b(L), lnf_g, lnf_b
        lnw = nc.dram_tensor("lnw", [4 * L + 2, D], F32, kind="ExternalInput")
    lm_w = nc.dram_tensor("lm_w", [NVS, P, KO, VS], BF16, kind="ExternalInput")

    # ---- per-core DRAM outputs ----
    logits = nc.dram_tensor("logits", [TL, V], F32, kind="ExternalOutput")
    lse = nc.dram_tensor("lse", [P, TT], F32, kind="ExternalOutput")

    groups = [[0, 1, 2, 3], [4, 5, 6, 7]]

    with tile.TileContext(nc) as tc:
        with (
            tc.tile_pool(name="sb", bufs=2) as sb,
            tc.tile_pool(name="ps", bufs=8, space="PSUM") as ps,
            tc.tile_pool(name="dram", bufs=2, space="DRAM") as dram,
        ):
            ones_bf = sb.tile([P, P], BF16, tag="ones", bufs=1, name="ones")
            nc.vector.memset(ones_bf[:], 1.0)
            ident = sb.tile([P, P], F32, tag="ident", bufs=1, name="ident")
            make_identity(nc, ident[:])
            mask_sb = sb.tile([P, 2 * GRP, TL], BF16, tag="mask", bufs=1,
                              name="mask_sb")
            eps_tile = sb.tile([P, 1], F32, tag="eps", bufs=1, name="eps_tile")
            nc.vector.memset(eps_tile[:], EPS)
            nc.sync.dma_start(mask_sb[:], maskT[:, :, :])

            lnsb = None
            if lnw is not None:
                lnsb = sb.tile([4 * L + 2, P, KO], F32, tag="lnsb", bufs=1,
                               name="lnsb")
                nc.sync.dma_start(
                    lnsb[:], lnw[:, :].rearrange("n (ko p) -> n p ko", p=P))

            def ln_gb(kind, layer):
                if lnsb is None or ln_trivial[kind]:
                    return None, None
                if kind == 0:
                    gr, br = layer, L + layer
                elif kind == 1:
                    gr, br = 2 * L + layer, 3 * L + layer
                else:
                    gr, br = 4 * L, 4 * L + 1
                return lnsb[gr], lnsb[br]

            # ---- residual stream, channel-major fp32 ----
            xT = sb.tile([P, KO, TL], F32, tag="xT", bufs=1, name="xT")

            # ---- embedding: gather + pos add (token-major), then transpose --
            for tt in range(TT):
                idx_sb = sb.tile([P, 1], I32, tag="idx", bufs=2, name="idx_sb")
                nc.sync.dma_start(idx_sb[:], idx[tt * P:(tt + 1) * P, None])
                emb = sb.tile([P, D], BF16, tag="hT", bufs=2, name="emb")
                nc.gpsimd.indirect_dma_start(
                    out=emb[:], out_offset=None, in_=tok_emb[:, :],
                    in_offset=bass.IndirectOffsetOnAxis(ap=idx_sb[:, :1], axis=0),
                )
                pos_sb = sb.tile([P, D], F32, tag="ln_xbf", bufs=1, name="pos_sb")
                nc.sync.dma_start(pos_sb[:], pos[tt * P:(tt + 1) * P, :])
                x0 = sb.tile([P, D], F32, tag="ln_xsq", bufs=1, name="x0")
                nc.vector.tensor_tensor(x0[:], emb[:], pos_sb[:], op=ALU.add)
                for ko in range(KO):
                    pst = ps.tile([P, 512], F32, tag="ps", bufs=8, name="pst")
                    nc.tensor.transpose(
                        pst[:, :P], x0[:, ko * P:(ko + 1) * P], ident[:])
                    nc.vector.tensor_copy(
                        xT[:, ko, tt * P:(tt + 1) * P], pst[:, :P])

            # ================= transformer layers =================
            for l in range(L):
                # -- LN1 --
                xn1 = sb.tile([P, KO, TL], BF16, tag="xn", bufs=2, name="xn1")
                g, b = ln_gb(0, l)
                _ln_channel_major(nc, sb, ps, xT, ones_bf, eps_tile, xn1, g, b)

                # -- attention weights (bf16, lhsT layout [p, ko, m]) --
                wq_sb = sb.tile([P, KO, D], BF16, tag="wq", bufs=1, name="wq_sb")
                wk_sb = sb.tile([P, KO, D], BF16, tag="wk", bufs=1, name="wk_sb")
                wv_sb = sb.tile([P, KO, D], BF16, tag="wv", bufs=1, name="wv_sb")
                nc.sync.dma_start(wq_sb[:], wq[l])
                nc.sync.dma_start(wk_sb[:], wk[l])
                nc.sync.dma_start(wv_sb[:], wv[l])

                # -- K^T channel-major, V token-major; gather early so the
                # collective overlaps the Q projection --
                qT = sb.tile([P, KO, TL], BF16, tag="qT", bufs=1, name="qT")
                kT = sb.tile([P, KO, TL], BF16, tag="kT", bufs=1, name="kT")
                for mp in range(KO // 2):
                    pq = ps.tile([P, 512], F32, tag="ps", bufs=8, name="pq")
                    for half in range(2):
                        m = 2 * mp + half
                        for ko in range(KO):
                            nc.tensor.matmul(
                                pq[:, half * TL:half * TL + TL],
                                lhsT=wk_sb[:, ko, m * P:(m + 1) * P],
                                rhs=xn1[:, ko, :],
                                start=(ko == 0), stop=(ko == KO - 1))
                    nc.scalar.activation(
                        kT[:, 2 * mp:2 * mp + 2, :].rearrange(
                            "p a b -> p (a b)"),
                        pq[:], AF.Copy)

                vtok = sb.tile([P, TT, D], BF16, tag="vtok", bufs=1, name="vtok")
                for tt in range(TT):
                    for dh in range(D // 512):
                        pv = ps.tile([P, 512], F32, tag="ps", bufs=8, name="pv")
                        for ko in range(KO):
                            nc.tensor.matmul(
                                pv[:],
                                lhsT=xn1[:, ko, tt * P:(tt + 1) * P],
                                rhs=wv_sb[:, ko, dh * 512:(dh + 1) * 512],
                                start=(ko == 0), stop=(ko == KO - 1))
                        nc.vector.tensor_copy(
                            vtok[:, tt, dh * 512:(dh + 1) * 512], pv[:])

                # -- K then V all-gather (split so S can start sooner) --
                k_in = dram.tile([P, KO * TL], BF16, tag="k_in", bufs=2,
                                 name="k_in")
                k_out = dram.tile([GRP, P, KO * TL], BF16, tag="k_out", bufs=2,
                                  name="k_out")
                v_in = dram.tile([P, TT * D], BF16, tag="v_in", bufs=2,
                                 name="v_in")
                v_out = dram.tile([GRP, P, TT * D], BF16, tag="v_out", bufs=2,
                                  name="v_out")
                nc.sync.dma_start(
                    k_in[:].rearrange("p (ko t) -> p ko t", t=TL), kT[:])
                nc.gpsimd.collective_compute(
                    "AllGather", ALU.bypass, replica_groups=groups,
                    ins=[k_in.opt()], outs=[k_out.opt()])
                nc.sync.dma_start(
                    v_in[:].rearrange("p (tt d) -> p tt d", d=D), vtok[:])
                nc.gpsimd.collective_compute(
                    "AllGather", ALU.bypass, replica_groups=groups,
                    ins=[v_in.opt()], outs=[v_out.opt()])
                kg = sb.tile([P, GRP, KO, TL], BF16, tag="kg", bufs=1, name="kg")
                vg = sb.tile([P, GRP, TT, D], BF16, tag="vg", bufs=1, name="vg")
                for s in range(GRP):
                    nc.sync.dma_start(
                        kg[:, s],
                        k_out[s].rearrange("p (ko t) -> p ko t", t=TL))
                    nc.sync.dma_start(
                        vg[:, s],
                        v_out[s].rearrange("p (tt d) -> p tt d", d=D))

                # -- Q^T (overlaps the gather) --
                for mp in range(KO // 2):
                    pq = ps.tile([P, 512], F32, tag="ps", bufs=8, name="pq")
                    for half in range(2):
                        m = 2 * mp + half
                        for ko in range(KO):
                            nc.tensor.matmul(
                                pq[:, half * TL:half * TL + TL],
                                lhsT=wq_sb[:, ko, m * P:(m + 1) * P],
                                rhs=xn1[:, ko, :],
                                start=(ko == 0), stop=(ko == KO - 1))
                    nc.scalar.activation(
                        qT[:, 2 * mp:2 * mp + 2, :].rearrange(
                            "p a b -> p (a b)"),
                        pq[:], AF.Copy, scale=float(1.0 / np.sqrt(HD)))

                # -- attention, one head pair at a time --
                yT = sb.tile([P, KO, TL], BF16, tag="yT", bufs=1, name="yT")
                for j in range(H // 2):          # heads a=2j (parts 0:64), b=2j+1
                    sps = {}
                    for hb in range(2):          # head in pair
                        hp = hb * HD
                        for i in range(4):
                            sps[hb, i] = ps.tile([P, 512], F32, tag="ps",
                                                 bufs=8, name=f"sps{hb}{i}")
                    for s in range(GRP):
                        for tt in range(TT):
                            kt = 2 * s + tt
                            for hb in range(2):  # adjacent row-groups overlap
                                hp = hb * HD
                                nc.tensor.matmul(
                                    sps[hb, kt // 2][:, (kt % 2) * TL:
                                                     (kt % 2) * TL + TL],
                                    lhsT=kg[hp:hp + HD, s, j,
                                            tt * P:(tt + 1) * P],
                                    rhs=qT[hp:hp + HD, j, :],
                                    start=True, stop=True)
                    expS = {}
                    for hb in range(2):
                        expS[hb] = sb.tile([P, 2 * GRP, TL], BF16, tag="expS",
                                           bufs=3, name=f"expS{hb}")
                        for i in range(4):
                            nc.scalar.activation(
                                expS[hb][:, 2 * i:2 * i + 2, :].rearrange(
                                    "p a b -> p (a b)"),
                                sps[hb, i][:], AF.Exp)
                            nc.vector.tensor_tensor(
                                expS[hb][:, 2 * i:2 * i + 2, :],
                                expS[hb][:, 2 * i:2 * i + 2, :],
                                mask_sb[:, 2 * i:2 * i + 2, :], op=ALU.mult)
                    # Z/Y for both heads packed into shared banks via
                    # sequential per-head chains on disjoint col groups
                    zp = ps.tile([P, 512], F32, tag="ps", bufs=8, name="zp")
                    for hb in range(2):
                        hp = hb * HD
                        for kt in range(2 * GRP):
                            nc.tensor.matmul(
                                zp[hp:hp + HD, :TL], lhsT=ones_bf[:, :HD],
                                rhs=expS[hb][:, kt, :],
                                start=(kt == 0), stop=(kt == 2 * GRP - 1))
                    yp = ps.tile([P, 512], F32, tag="ps", bufs=8, name="yp")
                    for hb in range(2):
                        hp = hb * HD
                        h = 2 * j + hb
                        for s in range(GRP):
                            for tt in range(TT):
                                kt = 2 * s + tt
                                nc.tensor.matmul(
                                    yp[hp:hp + HD, :TL],
                                    lhsT=vg[:, s, tt, h * HD:(h + 1) * HD],
                                    rhs=expS[hb][:, kt, :],
                                    start=(kt == 0), stop=(kt == 2 * GRP - 1))
                    rz = sb.tile([P, TL], F32, tag="rz", bufs=2, name="rz")
                    nc.vector.reciprocal_approx_fast(rz[:], zp[:, :TL])
                    nc.vector.tensor_tensor(
                        yT[:, j, :], yp[:, :TL], rz[:], op=ALU.mult)

                # -- out proj (reuses wv) + residual --
                for mp in range(KO // 2):
                    po = ps.tile([P, 512], F32, tag="ps", bufs=8, name="po")
                    for half in range(2):
                        m = 2 * mp + half
                        for ko in range(KO):
                            nc.tensor.matmul(
                                po[:, half * TL:half * TL + TL],
                                lhsT=wv_sb[:, ko, m * P:(m + 1) * P],
                                rhs=yT[:, ko, :],
                                start=(ko == 0), stop=(ko == KO - 1))
                    xflat = xT[:, 2 * mp:2 * mp + 2, :].rearrange(
                        "p a b -> p (a b)")
                    nc.vector.tensor_tensor(xflat, xflat, po[:], op=ALU.add)

                # -- LN2 --
                xn2 = sb.tile([P, KO, TL], BF16, tag="xn", bufs=2, name="xn2")
                g, b = ln_gb(1, l)
                _ln_channel_major(nc, sb, ps, xT, ones_bf, eps_tile, xn2, g, b)

                # -- MLP --
                # w1/gelu phase: materialize all of h (reuses the kg slot,
                # which is dead after the attention block)
                NCH = DH // 512
                h_full = sb.tile([P, DH // P, TL], BF16, tag="kg", bufs=1,
                                 name="h_full")
                for c in range(NCH):
                    w1c = sb.tile([P, KO, 512], BF16, tag="w1c", bufs=2,
                                  name="w1c")
                    nc.sync.dma_start(w1c[:], w1[l, c])
                    for mp in range(2):
                        ph = ps.tile([P, 512], F32, tag="ps", bufs=8, name="ph")
                        for half in range(2):
                            m = 2 * mp + half
                            for ko in range(KO):
                                nc.tensor.matmul(
                                    ph[:, half * TL:half * TL + TL],
                                    lhsT=w1c[:, ko, m * P:(m + 1) * P],
                                    rhs=xn2[:, ko, :],
                                    start=(ko == 0), stop=(ko == KO - 1))
                        nc.scalar.activation(
                            h_full[:, c * 4 + 2 * mp:c * 4 + 2 * mp + 2, :]
                            .rearrange("p a b -> p (a b)"),
                            ph[:], GELU)
                # w2 phase: two passes over output-channel halves; each pass
                # runs 4 bank-exclusive accumulation chains over all of h
                for hp2 in range(2):
                    mo = [ps.tile([P, 512], F32, tag="ps", bufs=8,
                                  name=f"mo{i}") for i in range(4)]
                    for c in range(NCH):
                        w2c = sb.tile([P, 4, 512], BF16, tag="w2c", bufs=2,
                                      name="w2c")
                        nc.sync.dma_start(w2c[:], w2[l, hp2, c])
                        for kc in range(4):
                            for m in range(4):
                                nc.tensor.matmul(
                                    mo[m][:, :TL],
                                    lhsT=w2c[:, kc, m * P:(m + 1) * P],
                                    rhs=h_full[:, c * 4 + kc, :],
                                    start=(c == 0 and kc == 0),
                                    stop=(c == NCH - 1 and kc == 3))
                    for m in range(4):
                        nc.vector.tensor_tensor(
                            xT[:, 4 * hp2 + m, :], xT[:, 4 * hp2 + m, :],
                            mo[m][:, :TL], op=ALU.add)

            # ================= final LN + LM head =================
            NVS = V // VS
            xnf = sb.tile([P, KO, TL], BF16, tag="xn", bufs=2, name="xnf")
            g, b = ln_gb(2, 0)
            _ln_channel_major(nc, sb, ps, xT, ones_bf, eps_tile, xnf, g, b)

            sumexp = sb.tile([P, TT], F32, tag="sumexp", bufs=1, name="sumexp")
            nc.vector.memset(sumexp[:], 0.0)
            for vs in range(NVS):
                lw = sb.tile([P, KO, VS], BF16, tag="lmw", bufs=3, name="lw")
                nc.sync.dma_start(lw[:], lm_w[vs])
                for tt in range(TT):
                    pl = ps.tile([P, 512], F32, tag="ps", bufs=8, name="pl")
                    for ko in range(KO):
                        nc.tensor.matmul(
                            pl[:, :VS], lhsT=xnf[:, ko, tt * P:(tt + 1) * P],
                            rhs=lw[:, ko, :],
                            start=(ko == 0), stop=(ko == KO - 1))
                    lsb = sb.tile([P, VS], F32, tag="lsb", bufs=2, name="lsb")
                    nc.vector.tensor_copy(lsb[:], pl[:, :VS])
                    nc.sync.dma_start(
                        logits[tt * P:(tt + 1) * P, vs * VS:(vs + 1) * VS],
                        lsb[:])
                    el = sb.tile([P, VS], F32, tag="el", bufs=2, name="el")
                    nc.scalar.activation(el[:], pl[:, :VS], AF.Exp)
                    se = sb.tile([P, 1], F32, tag="se", bufs=2, name="se")
                    nc.vector.tensor_reduce(se[:], el[:], axis=AX.X, op=ALU.add)
                    nc.vector.tensor_tensor(
                        sumexp[:, tt:tt + 1], sumexp[:, tt:tt + 1], se[:],
                        op=ALU.add)
            lse_sb = sb.tile([P, TT], F32, tag="lse", bufs=1, name="lse_sb")
            nc.scalar.activation(lse_sb[:], sumexp[:], AF.Ln)
            nc.sync.dma_start(lse[:, :], lse_sb[:])

    nc.compile()
    return nc


def _to_bf16(a):
    return np.ascontiguousarray(np.asarray(a).astype(ml_dtypes.bfloat16))


def make_in_maps(inp, ln_trivial):
    idx = np.asarray(inp["idx"]).astype(np.int32)
    pos_f = np.asarray(inp["pos_emb"]).astype(np.float32)
    tok_emb = _to_bf16(inp["tok_emb"])

    def prep_sq(w):  # [L, D, M] -> [L, P, KO, M]
        w = _to_bf16(w)
        Lw, Din, M = w.shape
        return np.ascontiguousarray(
            w.reshape(Lw, KO, P, M).transpose(0, 2, 1, 3))

    wq, wk, wv = prep_sq(inp["wq"]), prep_sq(inp["wk"]), prep_sq(inp["wv"])
    w1 = _to_bf16(inp["w1"])  # [L, D, DH] -> [L, NCH, P, KO, 512]
    w1 = np.ascontiguousarray(
        w1.reshape(L, KO, P, DH // 512, 512).transpose(0, 3, 2, 1, 4))
    w2 = _to_bf16(inp["w2"])  # [L, DH, D] -> [L, 2, NCH, P, 4, 512]
    w2 = np.ascontiguousarray(
        w2.reshape(L, DH // 512, 4, P, 2, 512).transpose(0, 4, 1, 3, 2, 5))
    lm_w = _to_bf16(inp["lm_w"])  # [D, V] -> [NVS, P, KO, VS]
    NVS = V // VS
    lm_w = np.ascontiguousarray(
        lm_w.reshape(KO, P, NVS, VS).transpose(2, 1, 0, 3))

    lnw = None
    if not all(ln_trivial):
        lnw = np.concatenate([
            np.asarray(inp["ln1_g"]), np.asarray(inp["ln1_b"]),
            np.asarray(inp["ln2_g"]), np.asarray(inp["ln2_b"]),
            np.asarray(inp["lnf_g"])[None], np.asarray(inp["lnf_b"])[None],
        ], axis=0).astype(np.float32)

    in_maps = []
    for c in range(NCORES):
        b, p = c // GRP, c % GRP
        tok0 = p * TL
        kp = np.arange(P)[:, None, None]
        kt = np.arange(2 * GRP)[None, :, None]
        q = np.arange(TL)[None, None, :]
        mask = ((kt * P + kp) <= (tok0 + q)).astype(ml_dtypes.bfloat16)
        m = {
            "tok_emb": tok_emb,
            "idx": np.ascontiguousarray(idx[b, tok0:tok0 + TL]),
            "pos": np.ascontiguousarray(pos_f[tok0:tok0 + TL]),
            "maskT": np.ascontiguousarray(mask),
            "wq": wq, "wk": wk, "wv": wv, "w1": w1, "w2": w2,
            "lm_w": lm_w,
        }
        if lnw is not None:
            m["lnw"] = lnw
        in_maps.append(m)
    return in_maps


def assemble(results, targets):
    logits = np.concatenate(
        [results[c]["logits"] for c in range(NCORES)], axis=0)
    lse = np.concatenate(
        [results[c]["lse"].T.reshape(-1) for c in range(NCORES)], axis=0)
    tgt = np.asarray(targets).reshape(-1).astype(np.int64)
    nll = lse.astype(np.float64) - logits[np.arange(len(tgt)), tgt].astype(np.float64)
    loss = np.float32(np.mean(nll))
    return logits, loss


def kernel(**inputs):
    inp = {k: np.asarray(v) for k, v in inputs.items()}
    for name in ("bq", "bk", "bv", "b1", "b2", "lm_b"):
        assert not np.any(inp[name]), f"{name} must be zero (unsupported)"

    ln_trivial = (
        bool(np.all(inp["ln1_g"] == 1) and not np.any(inp["ln1_b"])),
        bool(np.all(inp["ln2_g"] == 1) and not np.any(inp["ln2_b"])),
        bool(np.all(inp["lnf_g"] == 1) and not np.any(inp["lnf_b"])),
    )

    key = ("gpt", ln_trivial)
    if key not in _CACHED:
        _CACHED[key] = build_nc(ln_trivial)
    nc = _CACHED[key]

    in_maps = make_in_maps(inp, ln_trivial)
    res = run_bass_kernel_spmd(nc, in_maps, core_ids=list(range(NCORES)))
    return assemble(res.results, inp["targets"])


if __name__ == "__main__":
    build_nc((True, True, True))
    print("built ok")
